# revision 11
# baseline (speedup 1.0000x reference)
"""Trainium2 Bass kernel for the gated-attention MIL pooling layer.

Computes, for x:[256,128,1024], v,u:[1024,512], w:[512,1]:
    h = tanh(x @ v); g = sigmoid(x @ u)
    scores = (h*g) @ w                      # [256,128,1]
    alpha  = softmax(scores, axis=0)        # over the 256 instances

Sharding: data-parallel over the batch axis (128 -> 16 per core, 8 cores).

Mixed-precision matmuls chosen from measured end-to-end sensitivity
(the sigmoid path is ~3x less error-sensitive than the tanh path):

  - g = sigmoid(x @ u): fully fp8 e4m3 in DoubleRow perf mode (2 k-rows
    packed per PE cell, K=256 per instruction, 2x MAC rate).
  - h = tanh(x @ v), steady tiles: k-dims 0-255 via one DoubleRow fp8
    instruction + k-dims 256-1023 in bf16 (6 instructions).
  - chunks 0 and 1 (tiles 0-7): h fully fp8 as well, which shrinks the
    startup-critical DMA to ~2MB (fp8 x + fp8 weights) so the PE is
    never starved early (no HAM re-throttle) and ACT starts ~10us in.

All fp8/bf16 operands carry power-of-2 scales chosen so every h/g PSUM
accumulates S * 2^13 uniformly (x*16, v,u*512 for fp8; v*8192 for the
bf16 part), descaled exactly by the activation's free affine stage.
Measured end-to-end rel err ~1.6e-2 vs the 2e-2 gate.

Scheduling: raw Bass with explicit per-engine programs and standalone
wait_ge sync (the walrus build here rejects instructions with more than
one attached semaphore wait).  Chunk 0 streams per-k-subtile and runs
ko-outer so the PE starts on the first 128KB; chunk 1 runs q-outer so
each PSUM-bank-reuse wait clears just-in-time; steady chunks are
whole-slab double-buffered DMAs.  The DVE gate/score pipeline runs in
bf16 (2x DVE rate); softmax numerators use e^s = 1/sigmoid(-s) - 1 so
the ACT table set never switches (the exp table load costs ~2.7us).
Measured ~94.5us on 8 cores (f32r baseline: 147us), PE-bound at
~11 x 512-cycle matmul slots per 128-row tile.
"""

import numpy as np

N_INST, BATCH, IN_DIM, L_DIM = 256, 128, 1024, 512
N_CORES = 8
B_LOC = BATCH // N_CORES            # 16 batch elements per core
M = N_INST * B_LOC                  # 4096 rows per core
P = 128                             # SBUF partitions
KO8 = IN_DIM // (2 * P)             # 4 double-row fp8 subtiles (full k)
KF8 = 256                           # leading k-dims of steady h in fp8
KOB = (IN_DIM - KF8) // P           # 6 bf16 subtiles (steady h, k>=256)
MT = M // P                         # 32 m-tiles per core
MS = 4                              # m-tiles per x DMA chunk
NS = MT // MS                       # 8 DMA chunks

SX = 16.0                           # x fp8 scale (pow2: exact descale)
SV = 512.0                          # v/u fp8 scale
SB = SX * SV                        # bf16 v pre-scale (so PSUM is uniform)
DESCALE = 1.0 / (SX * SV)

_CACHE = {}


def _build_bass():
    from contextlib import ExitStack

    import concourse.bass as bass
    import concourse.mybir as mybir

    f32 = mybir.dt.float32
    bf16 = mybir.dt.bfloat16
    f8 = mybir.dt.float8e4
    DR = mybir.MatmulPerfMode.DoubleRow
    AF = mybir.ActivationFunctionType
    ALU = mybir.AluOpType

    nc = bass.Bass(
        trn_type="TRN2",
        target_bir_lowering=False,
        debug=False,
        enable_asserts=False,
    )

    # x8: [P, NS, KO8, 2, MS*P] fp8; (p, s, ko, j, mm) = x8[m, k=ko*256+j*128+p]
    x8 = nc.dram_tensor("x8", [P, NS, KO8, 2, MS * P], f8, kind="ExternalInput").ap()
    # xb: [P, NS, KOB, MS*P] bf16; (p, s, kb, mm) = x[m, k=KF8+kb*128+p]
    xb = nc.dram_tensor("xb", [P, NS, KOB, MS * P], bf16, kind="ExternalInput").ap()
    # v8/u8: [P, KO8, 2, L] fp8; (p, ko, j, l) = q[k=ko*256+j*128+p, l]
    v8 = nc.dram_tensor("v8", [P, KO8, 2, L_DIM], f8, kind="ExternalInput").ap()
    u8 = nc.dram_tensor("u8", [P, KO8, 2, L_DIM], f8, kind="ExternalInput").ap()
    # vb: [P, KOB, L] bf16, pre-scaled by SB; (p, kb, l) = v[k=KF8+kb*128+p, l]
    vb = nc.dram_tensor("vb", [P, KOB, L_DIM], bf16, kind="ExternalInput").ap()
    w_rep = nc.dram_tensor("w_rep", [P, L_DIM], bf16, kind="ExternalInput").ap()
    # selb[r, c] = (r%16 == c%16): one matmul turns the per-row exp sums
    # into per-batch softmax denominators broadcast back to all 128 rows.
    selb = nc.dram_tensor("selb", [P, P], bf16, kind="ExternalInput").ap()
    # out[r, t] = alpha of row m = t*128 + r (host transposes; no on-chip
    # transpose needed)
    out = nc.dram_tensor("out", [P, MT], f32, kind="ExternalOutput").ap()

    # s_pe tick after the h/g accumulation group of tile t finishes.
    # Chunks 0 and 1 run ko-outer (all four h groups complete, then all
    # four g); steady chunks alternate h/g per tile.
    def pe_h(t):
        return 8 * (t // MS) + t % MS + 1 if t < 2 * MS else 2 * t + 1

    def pe_g(t):
        return 8 * (t // MS) + t % MS + 5 if t < 2 * MS else 2 * t + 2

    # s_act tick after tanh/sigmoid of tile t.  For chunks 0/1 ACT runs
    # all four tanh then all four sigmoid (mirroring the PE's ko-outer
    # group completion order) so chunk 1's PSUM-bank-reuse waits clear
    # as early as possible; steady tiles alternate tanh/sigmoid.
    def act_tanh(t):
        return 8 * (t // MS) + t % MS + 1 if t < 2 * MS else 2 * t + 1

    def act_sig(t):
        return 8 * (t // MS) + t % MS + 5 if t < 2 * MS else 2 * t + 2

    ctx = ExitStack()
    with ctx:
        v8_sb = ctx.enter_context(nc.sbuf_tensor("v8_sb", [P, KO8, 2, L_DIM], f8))
        u8_sb = ctx.enter_context(nc.sbuf_tensor("u8_sb", [P, KO8, 2, L_DIM], f8))
        vb_sb = ctx.enter_context(nc.sbuf_tensor("vb_sb", [P, KOB, L_DIM], bf16))
        w_sb = ctx.enter_context(nc.sbuf_tensor("w_sb", [P, L_DIM], bf16))
        selb_sb = ctx.enter_context(nc.sbuf_tensor("selb_sb", [P, P], bf16))
        x8_sb = ctx.enter_context(
            nc.sbuf_tensor("x8_sb", [P, 2, KO8, 2, MS * P], f8)
        )
        xb_sb = ctx.enter_context(
            nc.sbuf_tensor("xb_sb", [P, 2, KOB, MS * P], bf16)
        )
        th_sb = ctx.enter_context(nc.sbuf_tensor("th_sb", [P, MS, L_DIM], bf16))
        sg_sb = ctx.enter_context(nc.sbuf_tensor("sg_sb", [P, MS, L_DIM], bf16))
        tw_sb = ctx.enter_context(nc.sbuf_tensor("tw_sb", [P, L_DIM], bf16))
        z_sb = ctx.enter_context(nc.sbuf_tensor("z_sb", [P, L_DIM], bf16))
        S_sb = ctx.enter_context(nc.sbuf_tensor("S_sb", [P, MT], f32))
        E_sb = ctx.enter_context(nc.sbuf_tensor("E_sb", [P, MT], f32))
        rsum_sb = ctx.enter_context(nc.sbuf_tensor("rsum_sb", [P, 1], f32))
        rsumb_sb = ctx.enter_context(nc.sbuf_tensor("rsumb_sb", [P, 1], bf16))
        recip_sb = ctx.enter_context(nc.sbuf_tensor("recip_sb", [P, 1], f32))
        alpha_sb = ctx.enter_context(nc.sbuf_tensor("alpha_sb", [P, MT], f32))
        warm_sb = ctx.enter_context(nc.sbuf_tensor("warm_sb", [P, 4], f32))

        # All 8 PSUM banks: 4 h accumulation groups + 4 g groups (slot t%4).
        h_ps = ctx.enter_context(nc.psum_tensor("h_ps", [P, MS, L_DIM], f32))
        g_ps = ctx.enter_context(nc.psum_tensor("g_ps", [P, MS, L_DIM], f32))
        # Epilogue PSUM aliases an h bank (dead by then; gated on s_dve).
        rep_ps = h_ps.ap()[:, 1, :1]         # [128, 1] per-batch denominators

        s_v8 = [ctx.enter_context(nc.semaphore(f"s_v8k{k}")) for k in range(KO8)]
        s_u8 = [ctx.enter_context(nc.semaphore(f"s_u8k{k}")) for k in range(KO8)]
        s_x80 = [ctx.enter_context(nc.semaphore(f"s_x80k{k}")) for k in range(KO8)]
        s_x81 = [ctx.enter_context(nc.semaphore(f"s_x81k{k}")) for k in range(KO8)]
        s_vb = ctx.enter_context(nc.semaphore("s_vb"))
        s_w = ctx.enter_context(nc.semaphore("s_w"))
        s_sel = ctx.enter_context(nc.semaphore("s_sel"))
        s_x = [ctx.enter_context(nc.semaphore(f"s_x{i}")) for i in range(NS)]
        s_out = ctx.enter_context(nc.semaphore("s_out"))
        s_pe = ctx.enter_context(nc.semaphore("s_pe"))
        s_act = ctx.enter_context(nc.semaphore("s_act"))
        s_dve = ctx.enter_context(nc.semaphore("s_dve"))

        block = ctx.enter_context(nc.Block())

        # Other tick conventions:
        #   s_pe epilogue: denominator matmul -> 65.
        #   s_act: sigmoid(-S) -> 65.
        #   s_dve: tile t: tw -> 3t+1, z -> 3t+2, reduce -> 3t+3 (96 after
        #          all); epilogue: recip(sig) -> 97, E/rsum -> 98,
        #          rsum bf16 copy -> 99, recip(den) -> 100, alpha -> 101.

        @block.sync
        def _(sync):
            # Startup stream in PE consumption order (all fp8, ~2.3MB):
            # (v8, x8 chunk0) per subtile, u8, w, x8 chunk1, vb, selb;
            # steady chunks as whole 1.25MB double-buffered DMAs.
            # Startup pieces span two ko-subtiles each: 2KB per partition
            # line.  1KB lines measure ~200GB/s vs ~360GB/s for 2KB+.
            for j in range(KO8 // 2):
                sync.dma_start(
                    v8_sb.ap()[:, 2 * j : 2 * j + 2, :, :],
                    v8[:, 2 * j : 2 * j + 2, :, :],
                ).then_inc(s_v8[j], 16)
                sync.dma_start(
                    x8_sb.ap()[:, 0, 2 * j : 2 * j + 2, :, :],
                    x8[:, 0, 2 * j : 2 * j + 2, :, :],
                ).then_inc(s_x80[j], 16)
            for j in range(KO8 // 2):
                sync.dma_start(
                    u8_sb.ap()[:, 2 * j : 2 * j + 2, :, :],
                    u8[:, 2 * j : 2 * j + 2, :, :],
                ).then_inc(s_u8[j], 16)
            sync.dma_start(w_sb.ap(), w_rep[:]).then_inc(s_w, 16)
            for j in range(KO8 // 2):
                sync.dma_start(
                    x8_sb.ap()[:, 1, 2 * j : 2 * j + 2, :, :],
                    x8[:, 1, 2 * j : 2 * j + 2, :, :],
                ).then_inc(s_x81[j], 16)
            sync.dma_start(vb_sb.ap(), vb[:]).then_inc(s_vb, 16)
            sync.dma_start(selb_sb.ap(), selb[:]).then_inc(s_sel, 16)
            for s in range(2, NS):
                # x slot s%2 free once PE finished chunk s-2
                sync.wait_ge(s_pe, 8 * (s - 1))
                sync.dma_start(
                    xb_sb.ap()[:, s % 2, :, :], xb[:, s, :, :]
                ).then_inc(s_x[s], 16)
                sync.dma_start(
                    x8_sb.ap()[:, s % 2, :, :, :], x8[:, s, :, :, :]
                ).then_inc(s_x[s], 16)
            sync.wait_ge(s_dve, 3 * MT + 5)
            sync.dma_start(out[:], alpha_sb.ap()).then_inc(s_out, 16)
            sync.wait_ge(s_out, 16)

        @block.tensor
        def _(tensor):
            # Warm-up: two fp32 broadcast matmuls (~1us each) keep the PE's
            # HAM activity window alive across the preamble->first-DMA dead
            # time (a gap there resets the 3.4us un-throttle window); real
            # matmuls then start right at data-ready and warm up quickly.
            c0 = nc.const_aps.aps[(f32, 0.0)]
            c0b = c0.to_broadcast((P, L_DIM))
            for j in range(2):
                nc.tensor.matmul(
                    g_ps.ap()[:1, j, :], c0, c0b, start=True, stop=True
                )

            def mm_dr(ps, q, ko, sb_slot, wt_sb):
                return nc.tensor.matmul(
                    ps.ap()[:, q, :],
                    x8_sb.ap()[:, sb_slot, ko, :, q * P : (q + 1) * P],
                    wt_sb.ap()[:, ko, :, :],
                    start=(ko == 0),
                    stop=(ko == KO8 - 1),
                    perf_mode=DR,
                )

            # ---- chunks 0 and 1: all-fp8 ----
            # Chunk 0 runs ko-outer (compute starts on the first DMA piece);
            # chunk 1 runs q-outer so each PSUM-bank-reuse wait (tanh/sig of
            # the chunk-0 tile in that bank) clears just-in-time per bank
            # instead of all four gating the first ko pass.
            for ko in range(KO8):
                if ko % 2 == 0:
                    tensor.wait_ge(s_v8[ko // 2], 16)
                    tensor.wait_ge(s_x80[ko // 2], 16)
                for q in range(MS):
                    mm = mm_dr(h_ps, q, ko, 0, v8_sb)
                    if ko == KO8 - 1:
                        mm.then_inc(s_pe, 1)  # ticks 1..4
            for ko in range(KO8):
                if ko % 2 == 0:
                    tensor.wait_ge(s_u8[ko // 2], 16)
                for q in range(MS):
                    mm = mm_dr(g_ps, q, ko, 0, u8_sb)
                    if ko == KO8 - 1:
                        mm.then_inc(s_pe, 1)  # ticks 5..8
            for q in range(MS):
                # h bank q free once tanh(q) done
                tensor.wait_ge(s_act, act_tanh(q))
                for ko in range(KO8):
                    if q == 0 and ko % 2 == 0:
                        tensor.wait_ge(s_x81[ko // 2], 16)
                    mm = mm_dr(h_ps, q, ko, 1, v8_sb)
                mm.then_inc(s_pe, 1)  # ticks 9..12
            for q in range(MS):
                # g bank q free once sigmoid(q) done
                tensor.wait_ge(s_act, act_sig(q))
                for ko in range(KO8):
                    mm = mm_dr(g_ps, q, ko, 1, u8_sb)
                mm.then_inc(s_pe, 1)  # ticks 13..16
            # ---- steady chunks: h = 1 fp8-DR (k<256) + 6 bf16; g = 4 DR ----
            tensor.wait_ge(s_vb, 16)
            for t in range(2 * MS, MT):
                s, q = divmod(t, MS)
                # h bank t%4 free once tanh(t-4) done
                tensor.wait_ge(s_act, act_tanh(t - MS))
                if q == 0:
                    tensor.wait_ge(s_x[s], 32)
                nc.tensor.matmul(
                    h_ps.ap()[:, t % MS, :],
                    x8_sb.ap()[:, s % 2, 0, :, q * P : (q + 1) * P],
                    v8_sb.ap()[:, 0, :, :],
                    start=True,
                    stop=False,
                    perf_mode=DR,
                )
                for kb in range(KOB):
                    mm = nc.tensor.matmul(
                        h_ps.ap()[:, t % MS, :],
                        xb_sb.ap()[:, s % 2, kb, q * P : (q + 1) * P],
                        vb_sb.ap()[:, kb, :],
                        start=False,
                        stop=(kb == KOB - 1),
                    )
                mm.then_inc(s_pe, 1)  # tick 2t+1
                # g bank t%4 free once sigmoid(t-4) done
                tensor.wait_ge(s_act, act_sig(t - MS))
                for ko in range(KO8):
                    mm = mm_dr(g_ps, t % MS, ko, s % 2, u8_sb)
                mm.then_inc(s_pe, 1)  # tick 2t+2
            # ---- epilogue ----
            tensor.wait_ge(s_sel, 16)
            tensor.wait_ge(s_dve, 3 * MT + 3)  # bf16 rsum ready; h banks dead
            nc.tensor.matmul(
                rep_ps, selb_sb.ap(), rsumb_sb.ap(), start=True, stop=True
            ).then_inc(s_pe, 1)  # -> 65: per-batch sums broadcast to rows

        @block.scalar
        def _(scalar):
            # Dummy activations: pre-load the tanh/sigmoid tables during the
            # DMA-bound startup.  No exp anywhere in this program: the
            # softmax numerators come from e^s = 1/sigmoid(-s) - 1, so the
            # ACT table set never switches (the exp set load costs ~2.7us
            # and would sit on the critical path right before the epilogue).
            c0 = nc.const_aps.aps[(f32, 0.0)]
            for j, fn in enumerate((AF.Tanh, AF.Sigmoid)):
                nc.scalar.activation(warm_sb.ap()[:, j : j + 1], c0, fn)

            def tanh_t(t):
                scalar.wait_ge(s_pe, pe_h(t))
                if t >= MS:
                    scalar.wait_ge(s_dve, 3 * (t - MS) + 1)  # th slot free
                nc.scalar.activation(
                    th_sb.ap()[:, t % MS, :],
                    h_ps.ap()[:, t % MS, :],
                    AF.Tanh,
                    scale=DESCALE,
                ).then_inc(s_act, 1)

            def sig_t(t):
                scalar.wait_ge(s_pe, pe_g(t))
                if t >= MS:
                    scalar.wait_ge(s_dve, 3 * (t - MS) + 2)  # sg slot free
                nc.scalar.activation(
                    sg_sb.ap()[:, t % MS, :],
                    g_ps.ap()[:, t % MS, :],
                    AF.Sigmoid,
                    scale=DESCALE,
                ).then_inc(s_act, 1)

            # Chunks 0/1: all-tanh then all-sigmoid per chunk (matches the
            # PE's ko-outer group completion order and act_tanh/act_sig).
            for c in range(2):
                for q in range(MS):
                    tanh_t(MS * c + q)
                for q in range(MS):
                    sig_t(MS * c + q)
            for t in range(2 * MS, MT):
                tanh_t(t)
                sig_t(t)
            # sigmoid(-S): numerators follow as 1/sig - 1 on the DVE (no
            # max-subtraction needed: |score| <= sum|w| ~ 28, well inside
            # fp32 range through the sigmoid/reciprocal round trip).
            scalar.wait_ge(s_dve, 3 * MT)  # S complete
            nc.scalar.activation(
                E_sb.ap(), S_sb.ap(), AF.Sigmoid, scale=-1.0
            ).then_inc(s_act, 1)  # -> 65

        @block.vector
        def _(vector):
            vector.wait_ge(s_w, 16)
            for t in range(MT):
                vector.wait_ge(s_act, act_tanh(t))
                nc.vector.tensor_tensor(
                    tw_sb.ap(), th_sb.ap()[:, t % MS, :], w_sb.ap(), ALU.mult
                ).then_inc(s_dve, 1)
                vector.wait_ge(s_act, act_sig(t))
                vector.wait_ge(s_dve, 3 * t + 1)  # tw RAW (same-engine order)
                nc.vector.tensor_tensor(
                    z_sb.ap(), tw_sb.ap(), sg_sb.ap()[:, t % MS, :], ALU.mult
                ).then_inc(s_dve, 1)
                vector.wait_ge(s_dve, 3 * t + 2)  # z RAW
                nc.vector.tensor_reduce(
                    S_sb.ap()[:, t : t + 1],
                    z_sb.ap(),
                    axis=mybir.AxisListType.X,
                    op=ALU.add,
                ).then_inc(s_dve, 1)
            # epilogue: E = 1/sigmoid(-S) - 1 (= e^S), rsum = row-sum(E)
            vector.wait_ge(s_act, 2 * MT + 1)  # sigmoid(-S) ready
            nc.vector.reciprocal(alpha_sb.ap(), E_sb.ap()).then_inc(
                s_dve, 1
            )  # 97 (alpha_sb as scratch)
            vector.wait_ge(s_dve, 3 * MT + 1)  # scratch RAW
            nc.vector.tensor_scalar(
                E_sb.ap(),
                alpha_sb.ap(),
                -1.0,
                None,
                op0=ALU.add,
                op1=ALU.add,
                accum_out=rsum_sb.ap(),
            ).then_inc(s_dve, 1)  # 98
            vector.wait_ge(s_dve, 3 * MT + 2)  # rsum RAW
            nc.vector.tensor_copy(rsumb_sb.ap(), rsum_sb.ap()).then_inc(
                s_dve, 1
            )  # 99: bf16 rsum so the selb matmul runs 1-pass (fp32 is 2-pass)
            vector.wait_ge(s_pe, 2 * MT + 1)  # rep_ps (denominators) ready
            nc.vector.reciprocal(recip_sb.ap(), rep_ps).then_inc(s_dve, 1)  # 100
            vector.wait_ge(s_dve, 3 * MT + 4)  # recip_sb RAW
            nc.vector.tensor_scalar_mul(
                alpha_sb.ap(), E_sb.ap(), recip_sb.ap()
            ).then_inc(s_dve, 1)  # 101

    return nc


def _host_inputs(x, v, u, w):
    """Build the per-core input maps (host-side shard + quantized layouts)."""
    import ml_dtypes

    f8 = ml_dtypes.float8_e4m3
    bf16 = ml_dtypes.bfloat16

    x = np.asarray(x, dtype=np.float32)
    v = np.asarray(v, dtype=np.float32)
    u = np.asarray(u, dtype=np.float32)
    w = np.asarray(w, dtype=np.float32).reshape(L_DIM)

    # fp8 weights: (p, ko, j, l) = q[k=ko*256+j*128+p, l]
    def wq8(a):
        return np.ascontiguousarray(
            (a * SV).astype(f8).reshape(KO8, 2, P, L_DIM).transpose(2, 0, 1, 3)
        )

    # bf16 v tail (k >= KF8), pre-scaled by SB: (p, kb, l) = v[KF8+kb*128+p, l]
    vbq = np.ascontiguousarray(
        (v[KF8:] * SB).astype(bf16).reshape(KOB, P, L_DIM).transpose(1, 0, 2)
    )

    w_rep = np.ascontiguousarray(np.broadcast_to(w, (P, L_DIM))).astype(bf16)
    selb = (
        np.arange(P)[:, None] % B_LOC == np.arange(P)[None, :] % B_LOC
    ).astype(bf16)

    common = {
        "v8": wq8(v), "u8": wq8(u), "vb": vbq,
        "w_rep": w_rep, "selb": selb,
    }
    xf8 = (x * SX).astype(f8)        # quantize once on the full tensor
    xbf = x[:, :, KF8:].astype(bf16)
    in_maps = []
    for c in range(N_CORES):
        sl = slice(c * B_LOC, (c + 1) * B_LOC)
        xc8 = xf8[:, sl, :].reshape(M, IN_DIM)
        xcb = xbf[:, sl, :].reshape(M, IN_DIM - KF8)
        # (p, s, ko, j, mm) = x8[m=s*512+mm, k=ko*256+j*128+p]
        x8c = np.ascontiguousarray(
            xc8.reshape(NS, MS * P, KO8, 2, P).transpose(4, 0, 2, 3, 1)
        )
        # (p, s, kb, mm) = x[m=s*512+mm, k=KF8+kb*128+p]
        xbc = np.ascontiguousarray(
            xcb.reshape(NS, MS * P, KOB, P).transpose(3, 0, 2, 1)
        )
        in_maps.append({"x8": x8c, "xb": xbc, **common})
    return in_maps


def kernel(x, v, u, w):
    from concourse.bass_utils import run_bass_kernel_spmd

    if "nc" not in _CACHE:
        _CACHE["nc"] = _build_bass()
    nc = _CACHE["nc"]

    in_maps = _host_inputs(x, v, u, w)
    res = run_bass_kernel_spmd(nc, in_maps, core_ids=list(range(N_CORES)))
    _CACHE["last_result"] = res

    parts = []
    for c in range(N_CORES):
        a = res.results[c]["out"]  # [128, 32]; out[r, t] -> m = t*128 + r
        parts.append(a.T.reshape(N_INST, B_LOC))
    full = np.concatenate(parts, axis=1)[:, :, None]
    return np.ascontiguousarray(full.astype(np.float32))


# revision 12
# speedup vs baseline: 1.1706x; 1.1706x over previous
"""Trainium2 Bass kernel for the gated-attention MIL pooling layer.

Computes, for x:[256,128,1024], v,u:[1024,512], w:[512,1]:
    h = tanh(x @ v); g = sigmoid(x @ u)
    scores = (h*g) @ w                      # [256,128,1]
    alpha  = softmax(scores, axis=0)        # over the 256 instances

Sharding: data-parallel over the batch axis (128 -> 16 per core, 8 cores).

Mixed-precision matmuls chosen from measured end-to-end sensitivity
(the sigmoid path is ~3x less error-sensitive than the tanh path):

  - g = sigmoid(x @ u): fully fp8 e4m3 in DoubleRow perf mode (2 k-rows
    packed per PE cell, K=256 per instruction, 2x MAC rate).
  - h = tanh(x @ v), steady tiles: k-dims 0-255 via one DoubleRow fp8
    instruction + k-dims 256-1023 in bf16 (6 instructions).
  - chunks 0 and 1 (tiles 0-7): h fully fp8 as well, which shrinks the
    startup-critical DMA to ~2MB (fp8 x + fp8 weights) so the PE is
    never starved early (no HAM re-throttle) and ACT starts ~10us in.

All fp8/bf16 operands carry power-of-2 scales chosen so every h/g PSUM
accumulates S * 2^13 uniformly (x*16, v,u*512 for fp8; v*8192 for the
bf16 part), descaled exactly by the activation's free affine stage.
Measured end-to-end rel err ~1.6e-2 vs the 2e-2 gate.

Scheduling: raw Bass with explicit per-engine programs and standalone
wait_ge sync (the walrus build here rejects instructions with more than
one attached semaphore wait).  Chunk 0 streams per-k-subtile and runs
ko-outer so the PE starts on the first 128KB; chunk 1 runs q-outer so
each PSUM-bank-reuse wait clears just-in-time; steady chunks are
whole-slab double-buffered DMAs.  The DVE gate/score pipeline runs in
bf16 (2x DVE rate); softmax numerators use e^s = 1/sigmoid(-s) - 1 so
the ACT table set never switches (the exp table load costs ~2.7us).
Measured ~94.5us on 8 cores (f32r baseline: 147us), PE-bound at
~11 x 512-cycle matmul slots per 128-row tile.
"""

import numpy as np

N_INST, BATCH, IN_DIM, L_DIM = 256, 128, 1024, 512
N_CORES = 8
B_LOC = BATCH // N_CORES            # 16 batch elements per core
M = N_INST * B_LOC                  # 4096 rows per core
P = 128                             # SBUF partitions
KO8 = IN_DIM // (2 * P)             # 4 double-row fp8 subtiles (full k)
KF8 = 256                           # leading k-dims of steady h in fp8
KOB = (IN_DIM - KF8) // P           # 6 bf16 subtiles (steady h, k>=256)
MT = M // P                         # 32 m-tiles per core
MS = 4                              # m-tiles per x DMA chunk
NS = MT // MS                       # 8 DMA chunks

SX = 16.0                           # x fp8 scale (pow2: exact descale)
SV = 512.0                          # v/u fp8 scale
SB = SX * SV                        # bf16 v pre-scale (so PSUM is uniform)
DESCALE = 1.0 / (SX * SV)

_CACHE = {}


def _build_bass():
    from contextlib import ExitStack

    import concourse.bass as bass
    import concourse.mybir as mybir

    f32 = mybir.dt.float32
    bf16 = mybir.dt.bfloat16
    f8 = mybir.dt.float8e4
    DR = mybir.MatmulPerfMode.DoubleRow
    AF = mybir.ActivationFunctionType
    ALU = mybir.AluOpType

    nc = bass.Bass(
        trn_type="TRN2",
        target_bir_lowering=False,
        debug=False,
        enable_asserts=False,
    )

    # x8: [P, NS, KO8, 2, MS*P] fp8; (p, s, ko, j, mm) = x8[m, k=ko*256+j*128+p]
    x8 = nc.dram_tensor("x8", [P, NS, KO8, 2, MS * P], f8, kind="ExternalInput").ap()
    # xb: [P, NS, KOB, MS*P] bf16; (p, s, kb, mm) = x[m, k=KF8+kb*128+p]
    xb = nc.dram_tensor("xb", [P, NS, KOB, MS * P], bf16, kind="ExternalInput").ap()
    # v8/u8: [P, KO8, 2, L] fp8; (p, ko, j, l) = q[k=ko*256+j*128+p, l]
    v8 = nc.dram_tensor("v8", [P, KO8, 2, L_DIM], f8, kind="ExternalInput").ap()
    u8 = nc.dram_tensor("u8", [P, KO8, 2, L_DIM], f8, kind="ExternalInput").ap()
    # vb: [P, KOB, L] bf16, pre-scaled by SB; (p, kb, l) = v[k=KF8+kb*128+p, l]
    vb = nc.dram_tensor("vb", [P, KOB, L_DIM], bf16, kind="ExternalInput").ap()
    w_rep = nc.dram_tensor("w_rep", [P, L_DIM], bf16, kind="ExternalInput").ap()
    # selb[r, c] = (r%16 == c%16): one matmul turns the per-row exp sums
    # into per-batch softmax denominators broadcast back to all 128 rows.
    selb = nc.dram_tensor("selb", [P, P], bf16, kind="ExternalInput").ap()
    # out[r, t] = alpha of row m = t*128 + r (host transposes; no on-chip
    # transpose needed)
    out = nc.dram_tensor("out", [P, MT], f32, kind="ExternalOutput").ap()

    # s_pe tick after the h/g accumulation group of tile t finishes.
    # Chunks 0 and 1 run ko-outer (all four h groups complete, then all
    # four g); steady chunks alternate h/g per tile.
    def pe_h(t):
        return 8 * (t // MS) + t % MS + 1 if t < 2 * MS else 2 * t + 1

    def pe_g(t):
        return 8 * (t // MS) + t % MS + 5 if t < 2 * MS else 2 * t + 2

    # s_act tick after tanh/sigmoid of tile t.  For chunks 0/1 ACT runs
    # all four tanh then all four sigmoid (mirroring the PE's ko-outer
    # group completion order) so chunk 1's PSUM-bank-reuse waits clear
    # as early as possible; steady tiles alternate tanh/sigmoid.
    def act_tanh(t):
        return 8 * (t // MS) + t % MS + 1 if t < 2 * MS else 2 * t + 1

    def act_sig(t):
        return 8 * (t // MS) + t % MS + 5 if t < 2 * MS else 2 * t + 2

    ctx = ExitStack()
    with ctx:
        v8_sb = ctx.enter_context(nc.sbuf_tensor("v8_sb", [P, KO8, 2, L_DIM], f8))
        u8_sb = ctx.enter_context(nc.sbuf_tensor("u8_sb", [P, KO8, 2, L_DIM], f8))
        vb_sb = ctx.enter_context(nc.sbuf_tensor("vb_sb", [P, KOB, L_DIM], bf16))
        w_sb = ctx.enter_context(nc.sbuf_tensor("w_sb", [P, L_DIM], bf16))
        selb_sb = ctx.enter_context(nc.sbuf_tensor("selb_sb", [P, P], bf16))
        x8_sb = ctx.enter_context(
            nc.sbuf_tensor("x8_sb", [P, 2, KO8, 2, MS * P], f8)
        )
        xb_sb = ctx.enter_context(
            nc.sbuf_tensor("xb_sb", [P, 2, KOB, MS * P], bf16)
        )
        th_sb = ctx.enter_context(nc.sbuf_tensor("th_sb", [P, MS, L_DIM], bf16))
        sg_sb = ctx.enter_context(nc.sbuf_tensor("sg_sb", [P, MS, L_DIM], bf16))
        tw_sb = ctx.enter_context(nc.sbuf_tensor("tw_sb", [P, L_DIM], bf16))
        z_sb = ctx.enter_context(nc.sbuf_tensor("z_sb", [P, L_DIM], bf16))
        S_sb = ctx.enter_context(nc.sbuf_tensor("S_sb", [P, MT], f32))
        E_sb = ctx.enter_context(nc.sbuf_tensor("E_sb", [P, MT], f32))
        rsum_sb = ctx.enter_context(nc.sbuf_tensor("rsum_sb", [P, 1], f32))
        rsumb_sb = ctx.enter_context(nc.sbuf_tensor("rsumb_sb", [P, 1], bf16))
        recip_sb = ctx.enter_context(nc.sbuf_tensor("recip_sb", [P, 1], f32))
        alpha_sb = ctx.enter_context(nc.sbuf_tensor("alpha_sb", [P, MT], f32))
        warm_sb = ctx.enter_context(nc.sbuf_tensor("warm_sb", [P, 4], f32))

        # All 8 PSUM banks: 4 h accumulation groups + 4 g groups (slot t%4).
        h_ps = ctx.enter_context(nc.psum_tensor("h_ps", [P, MS, L_DIM], f32))
        g_ps = ctx.enter_context(nc.psum_tensor("g_ps", [P, MS, L_DIM], f32))
        # Epilogue PSUM aliases an h bank (dead by then; gated on s_dve).
        rep_ps = h_ps.ap()[:, 1, :1]         # [128, 1] per-batch denominators

        s_v8 = [ctx.enter_context(nc.semaphore(f"s_v8k{k}")) for k in range(KO8)]
        s_u8 = [ctx.enter_context(nc.semaphore(f"s_u8k{k}")) for k in range(KO8)]
        s_x80 = [ctx.enter_context(nc.semaphore(f"s_x80k{k}")) for k in range(KO8)]
        s_x81 = [ctx.enter_context(nc.semaphore(f"s_x81k{k}")) for k in range(KO8)]
        s_vb = ctx.enter_context(nc.semaphore("s_vb"))
        s_w = ctx.enter_context(nc.semaphore("s_w"))
        s_sel = ctx.enter_context(nc.semaphore("s_sel"))
        s_x = [ctx.enter_context(nc.semaphore(f"s_x{i}")) for i in range(NS)]
        s_out = ctx.enter_context(nc.semaphore("s_out"))
        s_pe = ctx.enter_context(nc.semaphore("s_pe"))
        s_act = ctx.enter_context(nc.semaphore("s_act"))
        s_dve = ctx.enter_context(nc.semaphore("s_dve"))

        block = ctx.enter_context(nc.Block())

        # Other tick conventions:
        #   s_pe epilogue: denominator matmul -> 65.
        #   s_act: sigmoid(-S) -> 65.
        #   s_dve: tile t: tw -> 3t+1, z -> 3t+2, reduce -> 3t+3 (96 after
        #          all); epilogue: recip(sig) -> 97, E/rsum -> 98,
        #          rsum bf16 copy -> 99, recip(den) -> 100, alpha -> 101.

        @block.sync
        def _(sync):
            # Startup stream in PE consumption order (all fp8, ~2.3MB):
            # (v8, x8 chunk0) per subtile, u8, w, x8 chunk1, vb, selb;
            # steady chunks as whole 1.25MB double-buffered DMAs.
            # Startup pieces span two ko-subtiles each: 2KB per partition
            # line.  1KB lines measure ~200GB/s vs ~360GB/s for 2KB+.
            for j in range(KO8 // 2):
                sync.dma_start(
                    v8_sb.ap()[:, 2 * j : 2 * j + 2, :, :],
                    v8[:, 2 * j : 2 * j + 2, :, :],
                ).then_inc(s_v8[j], 16)
                sync.dma_start(
                    x8_sb.ap()[:, 0, 2 * j : 2 * j + 2, :, :],
                    x8[:, 0, 2 * j : 2 * j + 2, :, :],
                ).then_inc(s_x80[j], 16)
            for j in range(KO8 // 2):
                sync.dma_start(
                    u8_sb.ap()[:, 2 * j : 2 * j + 2, :, :],
                    u8[:, 2 * j : 2 * j + 2, :, :],
                ).then_inc(s_u8[j], 16)
            sync.dma_start(w_sb.ap(), w_rep[:]).then_inc(s_w, 16)
            for j in range(KO8 // 2):
                sync.dma_start(
                    x8_sb.ap()[:, 1, 2 * j : 2 * j + 2, :, :],
                    x8[:, 1, 2 * j : 2 * j + 2, :, :],
                ).then_inc(s_x81[j], 16)
            sync.dma_start(vb_sb.ap(), vb[:]).then_inc(s_vb, 16)
            sync.dma_start(selb_sb.ap(), selb[:]).then_inc(s_sel, 16)
            for s in range(2, NS):
                # x slot s%2 free once PE finished chunk s-2
                sync.wait_ge(s_pe, 8 * (s - 1))
                sync.dma_start(
                    xb_sb.ap()[:, s % 2, :, :], xb[:, s, :, :]
                ).then_inc(s_x[s], 16)
                sync.dma_start(
                    x8_sb.ap()[:, s % 2, :, :, :], x8[:, s, :, :, :]
                ).then_inc(s_x[s], 16)
            sync.wait_ge(s_dve, 3 * MT + 5)
            sync.dma_start(out[:], alpha_sb.ap()).then_inc(s_out, 16)
            sync.wait_ge(s_out, 16)

        @block.tensor
        def _(tensor):
            # Warm-up: two fp32 broadcast matmuls (~1us each) keep the PE's
            # HAM activity window alive across the preamble->first-DMA dead
            # time (a gap there resets the 3.4us un-throttle window); real
            # matmuls then start right at data-ready and warm up quickly.
            c0 = nc.const_aps.aps[(f32, 0.0)]
            c0b = c0.to_broadcast((P, L_DIM))
            for j in range(3):
                nc.tensor.matmul(
                    g_ps.ap()[:1, j, :], c0, c0b, start=True, stop=True
                )

            def mm_dr(ps, q, ko, sb_slot, wt_sb):
                return nc.tensor.matmul(
                    ps.ap()[:, q, :],
                    x8_sb.ap()[:, sb_slot, ko, :, q * P : (q + 1) * P],
                    wt_sb.ap()[:, ko, :, :],
                    start=(ko == 0),
                    stop=(ko == KO8 - 1),
                    perf_mode=DR,
                )

            # ---- chunks 0 and 1: all-fp8 ----
            # Chunk 0 runs ko-outer (compute starts on the first DMA piece);
            # chunk 1 runs q-outer so each PSUM-bank-reuse wait (tanh/sig of
            # the chunk-0 tile in that bank) clears just-in-time per bank
            # instead of all four gating the first ko pass.
            for ko in range(KO8):
                if ko % 2 == 0:
                    tensor.wait_ge(s_v8[ko // 2], 16)
                    tensor.wait_ge(s_x80[ko // 2], 16)
                for q in range(MS):
                    mm = mm_dr(h_ps, q, ko, 0, v8_sb)
                    if ko == KO8 - 1:
                        mm.then_inc(s_pe, 1)  # ticks 1..4
            for ko in range(KO8):
                if ko % 2 == 0:
                    tensor.wait_ge(s_u8[ko // 2], 16)
                for q in range(MS):
                    mm = mm_dr(g_ps, q, ko, 0, u8_sb)
                    if ko == KO8 - 1:
                        mm.then_inc(s_pe, 1)  # ticks 5..8
            for q in range(MS):
                # h bank q free once tanh(q) done
                tensor.wait_ge(s_act, act_tanh(q))
                for ko in range(KO8):
                    if q == 0 and ko % 2 == 0:
                        tensor.wait_ge(s_x81[ko // 2], 16)
                    mm = mm_dr(h_ps, q, ko, 1, v8_sb)
                mm.then_inc(s_pe, 1)  # ticks 9..12
            for q in range(MS):
                # g bank q free once sigmoid(q) done
                tensor.wait_ge(s_act, act_sig(q))
                for ko in range(KO8):
                    mm = mm_dr(g_ps, q, ko, 1, u8_sb)
                mm.then_inc(s_pe, 1)  # ticks 13..16
            # ---- steady chunks: h = 1 fp8-DR (k<256) + 6 bf16; g = 4 DR ----
            tensor.wait_ge(s_vb, 16)
            for t in range(2 * MS, MT):
                s, q = divmod(t, MS)
                # h bank t%4 free once tanh(t-4) done
                tensor.wait_ge(s_act, act_tanh(t - MS))
                if q == 0:
                    tensor.wait_ge(s_x[s], 32)
                nc.tensor.matmul(
                    h_ps.ap()[:, t % MS, :],
                    x8_sb.ap()[:, s % 2, 0, :, q * P : (q + 1) * P],
                    v8_sb.ap()[:, 0, :, :],
                    start=True,
                    stop=False,
                    perf_mode=DR,
                )
                for kb in range(KOB):
                    mm = nc.tensor.matmul(
                        h_ps.ap()[:, t % MS, :],
                        xb_sb.ap()[:, s % 2, kb, q * P : (q + 1) * P],
                        vb_sb.ap()[:, kb, :],
                        start=False,
                        stop=(kb == KOB - 1),
                    )
                mm.then_inc(s_pe, 1)  # tick 2t+1
                # g bank t%4 free once sigmoid(t-4) done
                tensor.wait_ge(s_act, act_sig(t - MS))
                for ko in range(KO8):
                    mm = mm_dr(g_ps, t % MS, ko, s % 2, u8_sb)
                mm.then_inc(s_pe, 1)  # tick 2t+2
            # ---- epilogue ----
            tensor.wait_ge(s_sel, 16)
            tensor.wait_ge(s_dve, 3 * MT + 3)  # bf16 rsum ready; h banks dead
            nc.tensor.matmul(
                rep_ps, selb_sb.ap(), rsumb_sb.ap(), start=True, stop=True
            ).then_inc(s_pe, 1)  # -> 65: per-batch sums broadcast to rows

        @block.scalar
        def _(scalar):
            # Dummy activations: pre-load the tanh/sigmoid tables during the
            # DMA-bound startup.  No exp anywhere in this program: the
            # softmax numerators come from e^s = 1/sigmoid(-s) - 1, so the
            # ACT table set never switches (the exp set load costs ~2.7us
            # and would sit on the critical path right before the epilogue).
            c0 = nc.const_aps.aps[(f32, 0.0)]
            for j, fn in enumerate((AF.Tanh, AF.Sigmoid)):
                nc.scalar.activation(warm_sb.ap()[:, j : j + 1], c0, fn)

            def tanh_t(t):
                scalar.wait_ge(s_pe, pe_h(t))
                if t >= MS:
                    scalar.wait_ge(s_dve, 3 * (t - MS) + 1)  # th slot free
                nc.scalar.activation(
                    th_sb.ap()[:, t % MS, :],
                    h_ps.ap()[:, t % MS, :],
                    AF.Tanh,
                    scale=DESCALE,
                ).then_inc(s_act, 1)

            def sig_t(t):
                scalar.wait_ge(s_pe, pe_g(t))
                if t >= MS:
                    scalar.wait_ge(s_dve, 3 * (t - MS) + 2)  # sg slot free
                nc.scalar.activation(
                    sg_sb.ap()[:, t % MS, :],
                    g_ps.ap()[:, t % MS, :],
                    AF.Sigmoid,
                    scale=DESCALE,
                ).then_inc(s_act, 1)

            # Chunks 0/1: all-tanh then all-sigmoid per chunk (matches the
            # PE's ko-outer group completion order and act_tanh/act_sig).
            for c in range(2):
                for q in range(MS):
                    tanh_t(MS * c + q)
                for q in range(MS):
                    sig_t(MS * c + q)
            for t in range(2 * MS, MT):
                tanh_t(t)
                sig_t(t)
            # sigmoid(-S): numerators follow as 1/sig - 1 on the DVE (no
            # max-subtraction needed: |score| <= sum|w| ~ 28, well inside
            # fp32 range through the sigmoid/reciprocal round trip).
            scalar.wait_ge(s_dve, 3 * MT)  # S complete
            nc.scalar.activation(
                E_sb.ap(), S_sb.ap(), AF.Sigmoid, scale=-1.0
            ).then_inc(s_act, 1)  # -> 65

        @block.vector
        def _(vector):
            vector.wait_ge(s_w, 16)
            for t in range(MT):
                vector.wait_ge(s_act, act_tanh(t))
                nc.vector.tensor_tensor(
                    tw_sb.ap(), th_sb.ap()[:, t % MS, :], w_sb.ap(), ALU.mult
                ).then_inc(s_dve, 1)
                vector.wait_ge(s_act, act_sig(t))
                vector.wait_ge(s_dve, 3 * t + 1)  # tw RAW (same-engine order)
                nc.vector.tensor_tensor(
                    z_sb.ap(), tw_sb.ap(), sg_sb.ap()[:, t % MS, :], ALU.mult
                ).then_inc(s_dve, 1)
                vector.wait_ge(s_dve, 3 * t + 2)  # z RAW
                nc.vector.tensor_reduce(
                    S_sb.ap()[:, t : t + 1],
                    z_sb.ap(),
                    axis=mybir.AxisListType.X,
                    op=ALU.add,
                ).then_inc(s_dve, 1)
            # epilogue: E = 1/sigmoid(-S) - 1 (= e^S), rsum = row-sum(E)
            vector.wait_ge(s_act, 2 * MT + 1)  # sigmoid(-S) ready
            nc.vector.reciprocal(alpha_sb.ap(), E_sb.ap()).then_inc(
                s_dve, 1
            )  # 97 (alpha_sb as scratch)
            vector.wait_ge(s_dve, 3 * MT + 1)  # scratch RAW
            nc.vector.tensor_scalar(
                E_sb.ap(),
                alpha_sb.ap(),
                -1.0,
                None,
                op0=ALU.add,
                op1=ALU.add,
                accum_out=rsum_sb.ap(),
            ).then_inc(s_dve, 1)  # 98
            vector.wait_ge(s_dve, 3 * MT + 2)  # rsum RAW
            nc.vector.tensor_copy(rsumb_sb.ap(), rsum_sb.ap()).then_inc(
                s_dve, 1
            )  # 99: bf16 rsum so the selb matmul runs 1-pass (fp32 is 2-pass)
            vector.wait_ge(s_pe, 2 * MT + 1)  # rep_ps (denominators) ready
            nc.vector.reciprocal(recip_sb.ap(), rep_ps).then_inc(s_dve, 1)  # 100
            vector.wait_ge(s_dve, 3 * MT + 4)  # recip_sb RAW
            nc.vector.tensor_scalar_mul(
                alpha_sb.ap(), E_sb.ap(), recip_sb.ap()
            ).then_inc(s_dve, 1)  # 101

    return nc


def _host_inputs(x, v, u, w):
    """Build the per-core input maps (host-side shard + quantized layouts)."""
    import ml_dtypes

    f8 = ml_dtypes.float8_e4m3
    bf16 = ml_dtypes.bfloat16

    x = np.asarray(x, dtype=np.float32)
    v = np.asarray(v, dtype=np.float32)
    u = np.asarray(u, dtype=np.float32)
    w = np.asarray(w, dtype=np.float32).reshape(L_DIM)

    # fp8 weights: (p, ko, j, l) = q[k=ko*256+j*128+p, l]
    def wq8(a):
        return np.ascontiguousarray(
            (a * SV).astype(f8).reshape(KO8, 2, P, L_DIM).transpose(2, 0, 1, 3)
        )

    # bf16 v tail (k >= KF8), pre-scaled by SB: (p, kb, l) = v[KF8+kb*128+p, l]
    vbq = np.ascontiguousarray(
        (v[KF8:] * SB).astype(bf16).reshape(KOB, P, L_DIM).transpose(1, 0, 2)
    )

    w_rep = np.ascontiguousarray(np.broadcast_to(w, (P, L_DIM))).astype(bf16)
    selb = (
        np.arange(P)[:, None] % B_LOC == np.arange(P)[None, :] % B_LOC
    ).astype(bf16)

    common = {
        "v8": wq8(v), "u8": wq8(u), "vb": vbq,
        "w_rep": w_rep, "selb": selb,
    }
    xf8 = (x * SX).astype(f8)        # quantize once on the full tensor
    xbf = x[:, :, KF8:].astype(bf16)
    in_maps = []
    for c in range(N_CORES):
        sl = slice(c * B_LOC, (c + 1) * B_LOC)
        xc8 = xf8[:, sl, :].reshape(M, IN_DIM)
        xcb = xbf[:, sl, :].reshape(M, IN_DIM - KF8)
        # (p, s, ko, j, mm) = x8[m=s*512+mm, k=ko*256+j*128+p]
        x8c = np.ascontiguousarray(
            xc8.reshape(NS, MS * P, KO8, 2, P).transpose(4, 0, 2, 3, 1)
        )
        # (p, s, kb, mm) = x[m=s*512+mm, k=KF8+kb*128+p]
        xbc = np.ascontiguousarray(
            xcb.reshape(NS, MS * P, KOB, P).transpose(3, 0, 2, 1)
        )
        in_maps.append({"x8": x8c, "xb": xbc, **common})
    return in_maps


def kernel(x, v, u, w):
    from concourse.bass_utils import run_bass_kernel_spmd

    if "nc" not in _CACHE:
        _CACHE["nc"] = _build_bass()
    nc = _CACHE["nc"]

    in_maps = _host_inputs(x, v, u, w)
    res = run_bass_kernel_spmd(nc, in_maps, core_ids=list(range(N_CORES)))
    _CACHE["last_result"] = res

    parts = []
    for c in range(N_CORES):
        a = res.results[c]["out"]  # [128, 32]; out[r, t] -> m = t*128 + r
        parts.append(a.T.reshape(N_INST, B_LOC))
    full = np.concatenate(parts, axis=1)[:, :, None]
    return np.ascontiguousarray(full.astype(np.float32))


# revision 13
# speedup vs baseline: 1.1715x; 1.0008x over previous
"""Trainium2 Bass kernel for the gated-attention MIL pooling layer.

Computes, for x:[256,128,1024], v,u:[1024,512], w:[512,1]:
    h = tanh(x @ v); g = sigmoid(x @ u)
    scores = (h*g) @ w                      # [256,128,1]
    alpha  = softmax(scores, axis=0)        # over the 256 instances

Sharding: data-parallel over the batch axis (128 -> 16 per core, 8 cores).

Mixed-precision matmuls chosen from measured end-to-end sensitivity
(the sigmoid path is ~3x less error-sensitive than the tanh path):

  - g = sigmoid(x @ u): fully fp8 e4m3 in DoubleRow perf mode (2 k-rows
    packed per PE cell, K=256 per instruction, 2x MAC rate).
  - h = tanh(x @ v), steady tiles: k-dims 0-255 via one DoubleRow fp8
    instruction + k-dims 256-1023 in bf16 (6 instructions).
  - chunks 0 and 1 (tiles 0-7): h fully fp8 as well, which shrinks the
    startup-critical DMA to ~2MB (fp8 x + fp8 weights) so the PE is
    never starved early (no HAM re-throttle) and ACT starts ~10us in.

All fp8/bf16 operands carry power-of-2 scales chosen so every h/g PSUM
accumulates S * 2^13 uniformly (x*16, v,u*512 for fp8; v*8192 for the
bf16 part), descaled exactly by the activation's free affine stage.
Measured end-to-end rel err ~1.6e-2 vs the 2e-2 gate.

Scheduling: raw Bass with explicit per-engine programs and standalone
wait_ge sync (the walrus build here rejects instructions with more than
one attached semaphore wait).  Chunk 0 streams per-k-subtile and runs
ko-outer so the PE starts on the first 128KB; chunk 1 runs q-outer so
each PSUM-bank-reuse wait clears just-in-time; steady chunks are
whole-slab double-buffered DMAs.  The DVE gate/score pipeline runs in
bf16 (2x DVE rate); softmax numerators use e^s = 1/sigmoid(-s) - 1 so
the ACT table set never switches (the exp table load costs ~2.7us).
Measured ~94.5us on 8 cores (f32r baseline: 147us), PE-bound at
~11 x 512-cycle matmul slots per 128-row tile.
"""

import numpy as np

N_INST, BATCH, IN_DIM, L_DIM = 256, 128, 1024, 512
N_CORES = 8
B_LOC = BATCH // N_CORES            # 16 batch elements per core
M = N_INST * B_LOC                  # 4096 rows per core
P = 128                             # SBUF partitions
KO8 = IN_DIM // (2 * P)             # 4 double-row fp8 subtiles (full k)
KF8 = 256                           # leading k-dims of steady h in fp8
KOB = (IN_DIM - KF8) // P           # 6 bf16 subtiles (steady h, k>=256)
MT = M // P                         # 32 m-tiles per core
MS = 4                              # m-tiles per x DMA chunk
NS = MT // MS                       # 8 DMA chunks

SX = 16.0                           # x fp8 scale (pow2: exact descale)
SV = 512.0                          # v/u fp8 scale
SB = SX * SV                        # bf16 v pre-scale (so PSUM is uniform)
DESCALE = 1.0 / (SX * SV)

_CACHE = {}


def _build_bass():
    from contextlib import ExitStack

    import concourse.bass as bass
    import concourse.mybir as mybir

    f32 = mybir.dt.float32
    bf16 = mybir.dt.bfloat16
    f8 = mybir.dt.float8e4
    DR = mybir.MatmulPerfMode.DoubleRow
    AF = mybir.ActivationFunctionType
    ALU = mybir.AluOpType

    nc = bass.Bass(
        trn_type="TRN2",
        target_bir_lowering=False,
        debug=False,
        enable_asserts=False,
    )

    # x8: [P, NS, KO8, 2, MS*P] fp8; (p, s, ko, j, mm) = x8[m, k=ko*256+j*128+p]
    x8 = nc.dram_tensor("x8", [P, NS, KO8, 2, MS * P], f8, kind="ExternalInput").ap()
    # xb: [P, NS, KOB, MS*P] bf16; (p, s, kb, mm) = x[m, k=KF8+kb*128+p]
    xb = nc.dram_tensor("xb", [P, NS, KOB, MS * P], bf16, kind="ExternalInput").ap()
    # v8/u8: [P, KO8, 2, L] fp8; (p, ko, j, l) = q[k=ko*256+j*128+p, l]
    v8 = nc.dram_tensor("v8", [P, KO8, 2, L_DIM], f8, kind="ExternalInput").ap()
    u8 = nc.dram_tensor("u8", [P, KO8, 2, L_DIM], f8, kind="ExternalInput").ap()
    # vb: [P, KOB, L] bf16, pre-scaled by SB; (p, kb, l) = v[k=KF8+kb*128+p, l]
    vb = nc.dram_tensor("vb", [P, KOB, L_DIM], bf16, kind="ExternalInput").ap()
    w_rep = nc.dram_tensor("w_rep", [P, L_DIM], bf16, kind="ExternalInput").ap()
    # selb[r, c] = (r%16 == c%16): one matmul turns the per-row exp sums
    # into per-batch softmax denominators broadcast back to all 128 rows.
    selb = nc.dram_tensor("selb", [P, P], bf16, kind="ExternalInput").ap()
    # out[r, t] = alpha of row m = t*128 + r (host transposes; no on-chip
    # transpose needed)
    out = nc.dram_tensor("out", [P, MT], f32, kind="ExternalOutput").ap()

    # s_pe tick after the h/g accumulation group of tile t finishes.
    # Chunks 0 and 1 run ko-outer (all four h groups complete, then all
    # four g); steady chunks alternate h/g per tile.
    def pe_h(t):
        return 8 * (t // MS) + t % MS + 1 if t < 2 * MS else 2 * t + 1

    def pe_g(t):
        return 8 * (t // MS) + t % MS + 5 if t < 2 * MS else 2 * t + 2

    # s_act tick after tanh/sigmoid of tile t.  For chunks 0/1 ACT runs
    # all four tanh then all four sigmoid (mirroring the PE's ko-outer
    # group completion order) so chunk 1's PSUM-bank-reuse waits clear
    # as early as possible; steady tiles alternate tanh/sigmoid.
    def act_tanh(t):
        return 8 * (t // MS) + t % MS + 1 if t < 2 * MS else 2 * t + 1

    def act_sig(t):
        return 8 * (t // MS) + t % MS + 5 if t < 2 * MS else 2 * t + 2

    ctx = ExitStack()
    with ctx:
        v8_sb = ctx.enter_context(nc.sbuf_tensor("v8_sb", [P, KO8, 2, L_DIM], f8))
        u8_sb = ctx.enter_context(nc.sbuf_tensor("u8_sb", [P, KO8, 2, L_DIM], f8))
        vb_sb = ctx.enter_context(nc.sbuf_tensor("vb_sb", [P, KOB, L_DIM], bf16))
        w_sb = ctx.enter_context(nc.sbuf_tensor("w_sb", [P, L_DIM], bf16))
        selb_sb = ctx.enter_context(nc.sbuf_tensor("selb_sb", [P, P], bf16))
        x8_sb = ctx.enter_context(
            nc.sbuf_tensor("x8_sb", [P, 2, KO8, 2, MS * P], f8)
        )
        xb_sb = ctx.enter_context(
            nc.sbuf_tensor("xb_sb", [P, 2, KOB, MS * P], bf16)
        )
        th_sb = ctx.enter_context(nc.sbuf_tensor("th_sb", [P, MS, L_DIM], bf16))
        sg_sb = ctx.enter_context(nc.sbuf_tensor("sg_sb", [P, MS, L_DIM], bf16))
        tw_sb = ctx.enter_context(nc.sbuf_tensor("tw_sb", [P, L_DIM], bf16))
        z_sb = ctx.enter_context(nc.sbuf_tensor("z_sb", [P, L_DIM], bf16))
        S_sb = ctx.enter_context(nc.sbuf_tensor("S_sb", [P, MT], f32))
        E_sb = ctx.enter_context(nc.sbuf_tensor("E_sb", [P, MT], f32))
        rsum_sb = ctx.enter_context(nc.sbuf_tensor("rsum_sb", [P, 1], f32))
        rsumb_sb = ctx.enter_context(nc.sbuf_tensor("rsumb_sb", [P, 1], bf16))
        recip_sb = ctx.enter_context(nc.sbuf_tensor("recip_sb", [P, 1], f32))
        alpha_sb = ctx.enter_context(nc.sbuf_tensor("alpha_sb", [P, MT], f32))
        warm_sb = ctx.enter_context(nc.sbuf_tensor("warm_sb", [P, 4], f32))

        # All 8 PSUM banks: 4 h accumulation groups + 4 g groups (slot t%4).
        h_ps = ctx.enter_context(nc.psum_tensor("h_ps", [P, MS, L_DIM], f32))
        g_ps = ctx.enter_context(nc.psum_tensor("g_ps", [P, MS, L_DIM], f32))
        # Epilogue PSUM aliases an h bank (dead by then; gated on s_dve).
        rep_ps = h_ps.ap()[:, 1, :1]         # [128, 1] per-batch denominators

        s_v8 = [ctx.enter_context(nc.semaphore(f"s_v8k{k}")) for k in range(KO8)]
        s_u8 = [ctx.enter_context(nc.semaphore(f"s_u8k{k}")) for k in range(KO8)]
        s_x80 = [ctx.enter_context(nc.semaphore(f"s_x80k{k}")) for k in range(KO8)]
        s_x81 = [ctx.enter_context(nc.semaphore(f"s_x81k{k}")) for k in range(KO8)]
        s_vb = ctx.enter_context(nc.semaphore("s_vb"))
        s_w = ctx.enter_context(nc.semaphore("s_w"))
        s_sel = ctx.enter_context(nc.semaphore("s_sel"))
        s_x = [ctx.enter_context(nc.semaphore(f"s_x{i}")) for i in range(NS)]
        s_out = ctx.enter_context(nc.semaphore("s_out"))
        s_pe = ctx.enter_context(nc.semaphore("s_pe"))
        s_act = ctx.enter_context(nc.semaphore("s_act"))
        s_dve = ctx.enter_context(nc.semaphore("s_dve"))

        block = ctx.enter_context(nc.Block())

        # Other tick conventions:
        #   s_pe epilogue: denominator matmul -> 65.
        #   s_act: sigmoid(-S) -> 65.
        #   s_dve: tile t: tw -> 3t+1, z -> 3t+2, reduce -> 3t+3 (96 after
        #          all); epilogue: recip(sig) -> 97, E/rsum -> 98,
        #          rsum bf16 copy -> 99, recip(den) -> 100, alpha -> 101.

        @block.sync
        def _(sync):
            # Startup stream in PE consumption order (all fp8, ~2.3MB):
            # (v8, x8 chunk0) per subtile, u8, w, x8 chunk1, vb, selb;
            # steady chunks as whole 1.25MB double-buffered DMAs.
            # Startup pieces span two ko-subtiles each: 2KB per partition
            # line.  1KB lines measure ~200GB/s vs ~360GB/s for 2KB+.
            for j in range(KO8 // 2):
                sync.dma_start(
                    v8_sb.ap()[:, 2 * j : 2 * j + 2, :, :],
                    v8[:, 2 * j : 2 * j + 2, :, :],
                ).then_inc(s_v8[j], 16)
                sync.dma_start(
                    x8_sb.ap()[:, 0, 2 * j : 2 * j + 2, :, :],
                    x8[:, 0, 2 * j : 2 * j + 2, :, :],
                ).then_inc(s_x80[j], 16)
            for j in range(KO8 // 2):
                sync.dma_start(
                    u8_sb.ap()[:, 2 * j : 2 * j + 2, :, :],
                    u8[:, 2 * j : 2 * j + 2, :, :],
                ).then_inc(s_u8[j], 16)
            sync.dma_start(w_sb.ap(), w_rep[:]).then_inc(s_w, 16)
            for j in range(KO8 // 2):
                sync.dma_start(
                    x8_sb.ap()[:, 1, 2 * j : 2 * j + 2, :, :],
                    x8[:, 1, 2 * j : 2 * j + 2, :, :],
                ).then_inc(s_x81[j], 16)
            sync.dma_start(vb_sb.ap(), vb[:]).then_inc(s_vb, 16)
            sync.dma_start(selb_sb.ap(), selb[:]).then_inc(s_sel, 16)
            for s in range(2, NS):
                # x slot s%2 free once PE finished chunk s-2
                sync.wait_ge(s_pe, 8 * (s - 1))
                sync.dma_start(
                    xb_sb.ap()[:, s % 2, :, :], xb[:, s, :, :]
                ).then_inc(s_x[s], 16)
                sync.dma_start(
                    x8_sb.ap()[:, s % 2, :, :, :], x8[:, s, :, :, :]
                ).then_inc(s_x[s], 16)
            sync.wait_ge(s_dve, 3 * MT + 5)
            sync.dma_start(out[:], alpha_sb.ap()).then_inc(s_out, 16)
            sync.wait_ge(s_out, 16)

        @block.tensor
        def _(tensor):
            # Warm-up: two fp32 broadcast matmuls (~1us each) keep the PE's
            # HAM activity window alive across the preamble->first-DMA dead
            # time (a gap there resets the 3.4us un-throttle window); real
            # matmuls then start right at data-ready and warm up quickly.
            c0 = nc.const_aps.aps[(f32, 0.0)]
            c0b = c0.to_broadcast((P, L_DIM))
            for j in range(2):
                nc.tensor.matmul(
                    g_ps.ap()[:1, j, :], c0, c0b, start=True, stop=True
                )

            def mm_dr(ps, q, ko, sb_slot, wt_sb):
                return nc.tensor.matmul(
                    ps.ap()[:, q, :],
                    x8_sb.ap()[:, sb_slot, ko, :, q * P : (q + 1) * P],
                    wt_sb.ap()[:, ko, :, :],
                    start=(ko == 0),
                    stop=(ko == KO8 - 1),
                    perf_mode=DR,
                )

            # ---- chunks 0 and 1: all-fp8 ----
            # Chunk 0 runs ko-outer (compute starts on the first DMA piece);
            # chunk 1 runs q-outer so each PSUM-bank-reuse wait (tanh/sig of
            # the chunk-0 tile in that bank) clears just-in-time per bank
            # instead of all four gating the first ko pass.
            for ko in range(KO8):
                if ko % 2 == 0:
                    tensor.wait_ge(s_v8[ko // 2], 16)
                    tensor.wait_ge(s_x80[ko // 2], 16)
                for q in range(MS):
                    mm = mm_dr(h_ps, q, ko, 0, v8_sb)
                    if ko == KO8 - 1:
                        mm.then_inc(s_pe, 1)  # ticks 1..4
            for ko in range(KO8):
                if ko % 2 == 0:
                    tensor.wait_ge(s_u8[ko // 2], 16)
                for q in range(MS):
                    mm = mm_dr(g_ps, q, ko, 0, u8_sb)
                    if ko == KO8 - 1:
                        mm.then_inc(s_pe, 1)  # ticks 5..8
            for q in range(MS):
                # h bank q free once tanh(q) done
                tensor.wait_ge(s_act, act_tanh(q))
                for ko in range(KO8):
                    if q == 0 and ko % 2 == 0:
                        tensor.wait_ge(s_x81[ko // 2], 16)
                    mm = mm_dr(h_ps, q, ko, 1, v8_sb)
                mm.then_inc(s_pe, 1)  # ticks 9..12
            for q in range(MS):
                # g bank q free once sigmoid(q) done
                tensor.wait_ge(s_act, act_sig(q))
                for ko in range(KO8):
                    mm = mm_dr(g_ps, q, ko, 1, u8_sb)
                mm.then_inc(s_pe, 1)  # ticks 13..16
            # ---- steady chunks: h = 1 fp8-DR (k<256) + 6 bf16; g = 4 DR ----
            tensor.wait_ge(s_vb, 16)
            for t in range(2 * MS, MT):
                s, q = divmod(t, MS)
                # h bank t%4 free once tanh(t-4) done
                tensor.wait_ge(s_act, act_tanh(t - MS))
                if q == 0:
                    tensor.wait_ge(s_x[s], 32)
                nc.tensor.matmul(
                    h_ps.ap()[:, t % MS, :],
                    x8_sb.ap()[:, s % 2, 0, :, q * P : (q + 1) * P],
                    v8_sb.ap()[:, 0, :, :],
                    start=True,
                    stop=False,
                    perf_mode=DR,
                )
                for kb in range(KOB):
                    mm = nc.tensor.matmul(
                        h_ps.ap()[:, t % MS, :],
                        xb_sb.ap()[:, s % 2, kb, q * P : (q + 1) * P],
                        vb_sb.ap()[:, kb, :],
                        start=False,
                        stop=(kb == KOB - 1),
                    )
                mm.then_inc(s_pe, 1)  # tick 2t+1
                # g bank t%4 free once sigmoid(t-4) done
                tensor.wait_ge(s_act, act_sig(t - MS))
                for ko in range(KO8):
                    mm = mm_dr(g_ps, t % MS, ko, s % 2, u8_sb)
                mm.then_inc(s_pe, 1)  # tick 2t+2
            # ---- epilogue ----
            tensor.wait_ge(s_sel, 16)
            tensor.wait_ge(s_dve, 3 * MT + 3)  # bf16 rsum ready; h banks dead
            nc.tensor.matmul(
                rep_ps, selb_sb.ap(), rsumb_sb.ap(), start=True, stop=True
            ).then_inc(s_pe, 1)  # -> 65: per-batch sums broadcast to rows

        @block.scalar
        def _(scalar):
            # Dummy activations: pre-load the tanh/sigmoid tables during the
            # DMA-bound startup.  No exp anywhere in this program: the
            # softmax numerators come from e^s = 1/sigmoid(-s) - 1, so the
            # ACT table set never switches (the exp set load costs ~2.7us
            # and would sit on the critical path right before the epilogue).
            c0 = nc.const_aps.aps[(f32, 0.0)]
            for j, fn in enumerate((AF.Tanh, AF.Sigmoid)):
                nc.scalar.activation(warm_sb.ap()[:, j : j + 1], c0, fn)

            def tanh_t(t):
                scalar.wait_ge(s_pe, pe_h(t))
                if t >= MS:
                    scalar.wait_ge(s_dve, 3 * (t - MS) + 1)  # th slot free
                nc.scalar.activation(
                    th_sb.ap()[:, t % MS, :],
                    h_ps.ap()[:, t % MS, :],
                    AF.Tanh,
                    scale=DESCALE,
                ).then_inc(s_act, 1)

            def sig_t(t):
                scalar.wait_ge(s_pe, pe_g(t))
                if t >= MS:
                    scalar.wait_ge(s_dve, 3 * (t - MS) + 2)  # sg slot free
                nc.scalar.activation(
                    sg_sb.ap()[:, t % MS, :],
                    g_ps.ap()[:, t % MS, :],
                    AF.Sigmoid,
                    scale=DESCALE,
                ).then_inc(s_act, 1)

            # Chunks 0/1: all-tanh then all-sigmoid per chunk (matches the
            # PE's ko-outer group completion order and act_tanh/act_sig).
            for c in range(2):
                for q in range(MS):
                    tanh_t(MS * c + q)
                for q in range(MS):
                    sig_t(MS * c + q)
            for t in range(2 * MS, MT):
                tanh_t(t)
                sig_t(t)
            # sigmoid(-S): numerators follow as 1/sig - 1 on the DVE (no
            # max-subtraction needed: |score| <= sum|w| ~ 28, well inside
            # fp32 range through the sigmoid/reciprocal round trip).
            scalar.wait_ge(s_dve, 3 * MT)  # S complete
            nc.scalar.activation(
                E_sb.ap(), S_sb.ap(), AF.Sigmoid, scale=-1.0
            ).then_inc(s_act, 1)  # -> 65

        @block.vector
        def _(vector):
            vector.wait_ge(s_w, 16)
            for t in range(MT):
                vector.wait_ge(s_act, act_tanh(t))
                nc.vector.tensor_tensor(
                    tw_sb.ap(), th_sb.ap()[:, t % MS, :], w_sb.ap(), ALU.mult
                ).then_inc(s_dve, 1)
                vector.wait_ge(s_act, act_sig(t))
                vector.wait_ge(s_dve, 3 * t + 1)  # tw RAW (same-engine order)
                nc.vector.tensor_tensor(
                    z_sb.ap(), tw_sb.ap(), sg_sb.ap()[:, t % MS, :], ALU.mult
                ).then_inc(s_dve, 1)
                vector.wait_ge(s_dve, 3 * t + 2)  # z RAW
                nc.vector.tensor_reduce(
                    S_sb.ap()[:, t : t + 1],
                    z_sb.ap(),
                    axis=mybir.AxisListType.X,
                    op=ALU.add,
                ).then_inc(s_dve, 1)
            # epilogue: E = 1/sigmoid(-S) - 1 (= e^S), rsum = row-sum(E)
            vector.wait_ge(s_act, 2 * MT + 1)  # sigmoid(-S) ready
            nc.vector.reciprocal(alpha_sb.ap(), E_sb.ap()).then_inc(
                s_dve, 1
            )  # 97 (alpha_sb as scratch)
            vector.wait_ge(s_dve, 3 * MT + 1)  # scratch RAW
            nc.vector.tensor_scalar(
                E_sb.ap(),
                alpha_sb.ap(),
                -1.0,
                None,
                op0=ALU.add,
                op1=ALU.add,
                accum_out=rsum_sb.ap(),
            ).then_inc(s_dve, 1)  # 98
            vector.wait_ge(s_dve, 3 * MT + 2)  # rsum RAW
            nc.vector.tensor_copy(rsumb_sb.ap(), rsum_sb.ap()).then_inc(
                s_dve, 1
            )  # 99: bf16 rsum so the selb matmul runs 1-pass (fp32 is 2-pass)
            vector.wait_ge(s_pe, 2 * MT + 1)  # rep_ps (denominators) ready
            nc.vector.reciprocal(recip_sb.ap(), rep_ps).then_inc(s_dve, 1)  # 100
            vector.wait_ge(s_dve, 3 * MT + 4)  # recip_sb RAW
            nc.vector.tensor_scalar_mul(
                alpha_sb.ap(), E_sb.ap(), recip_sb.ap()
            ).then_inc(s_dve, 1)  # 101

    return nc


def _host_inputs(x, v, u, w):
    """Build the per-core input maps (host-side shard + quantized layouts)."""
    import ml_dtypes

    f8 = ml_dtypes.float8_e4m3
    bf16 = ml_dtypes.bfloat16

    x = np.asarray(x, dtype=np.float32)
    v = np.asarray(v, dtype=np.float32)
    u = np.asarray(u, dtype=np.float32)
    w = np.asarray(w, dtype=np.float32).reshape(L_DIM)

    # fp8 weights: (p, ko, j, l) = q[k=ko*256+j*128+p, l]
    def wq8(a):
        return np.ascontiguousarray(
            (a * SV).astype(f8).reshape(KO8, 2, P, L_DIM).transpose(2, 0, 1, 3)
        )

    # bf16 v tail (k >= KF8), pre-scaled by SB: (p, kb, l) = v[KF8+kb*128+p, l]
    vbq = np.ascontiguousarray(
        (v[KF8:] * SB).astype(bf16).reshape(KOB, P, L_DIM).transpose(1, 0, 2)
    )

    w_rep = np.ascontiguousarray(np.broadcast_to(w, (P, L_DIM))).astype(bf16)
    selb = (
        np.arange(P)[:, None] % B_LOC == np.arange(P)[None, :] % B_LOC
    ).astype(bf16)

    common = {
        "v8": wq8(v), "u8": wq8(u), "vb": vbq,
        "w_rep": w_rep, "selb": selb,
    }
    xf8 = (x * SX).astype(f8)        # quantize once on the full tensor
    xbf = x[:, :, KF8:].astype(bf16)
    in_maps = []
    for c in range(N_CORES):
        sl = slice(c * B_LOC, (c + 1) * B_LOC)
        xc8 = xf8[:, sl, :].reshape(M, IN_DIM)
        xcb = xbf[:, sl, :].reshape(M, IN_DIM - KF8)
        # (p, s, ko, j, mm) = x8[m=s*512+mm, k=ko*256+j*128+p]
        x8c = np.ascontiguousarray(
            xc8.reshape(NS, MS * P, KO8, 2, P).transpose(4, 0, 2, 3, 1)
        )
        # (p, s, kb, mm) = x[m=s*512+mm, k=KF8+kb*128+p]
        xbc = np.ascontiguousarray(
            xcb.reshape(NS, MS * P, KOB, P).transpose(3, 0, 2, 1)
        )
        in_maps.append({"x8": x8c, "xb": xbc, **common})
    return in_maps


def kernel(x, v, u, w):
    from concourse.bass_utils import run_bass_kernel_spmd

    if "nc" not in _CACHE:
        _CACHE["nc"] = _build_bass()
    nc = _CACHE["nc"]

    in_maps = _host_inputs(x, v, u, w)
    res = run_bass_kernel_spmd(nc, in_maps, core_ids=list(range(N_CORES)))
    _CACHE["last_result"] = res

    parts = []
    for c in range(N_CORES):
        a = res.results[c]["out"]  # [128, 32]; out[r, t] -> m = t*128 + r
        parts.append(a.T.reshape(N_INST, B_LOC))
    full = np.concatenate(parts, axis=1)[:, :, None]
    return np.ascontiguousarray(full.astype(np.float32))
